# revision 23
# baseline (speedup 1.0000x reference)
"""Trainium2 Bass kernel for nn_Block_10024453669245 (dense transformer block).

Strategy (8 NeuronCores), v2:
  - warmup: dummy 32B AllGather (high priority, first gpsimd inst) prepays
    communicator init.
  - Phase B: QKV tensor-parallel over heads (2 heads/core), bf16 matmuls.
    LN1 stats for the core's own 512 tokens are computed INSIDE block 0 of
    the QKV loop (ones-matmuls on xt_own tiles), then a tiny AllGather
    distributes (rstd, mu*rstd) rows. First RAW blocks evict uncorrected
    and are fixed up afterwards; later blocks fold LN1 via a K=1 matmul
    (rank-1 term) + eviction multiply against a PE-broadcast rstd row.
    The 4 v-chains share 2 PSUM banks (start=False overwrite trick).
  - Phase C: causal attention head-major, no-max-sub softmax, S^T tiles,
    exp on ACT, causal masks on diagonal tiles, O^T accumulated on PE.
    Softmax denominator via DVE accumulation of exp tiles + ONE ones-matmul
    per (b,tb) block; reciprocal broadcast across partitions via a PE
    ones-matmul (no DRAM round-trip). Per-head AllToAll (2MB) fires as
    soon as that head's outputs are done.
  - Phase E: MLP token-sharded (512 tokens/core) in bf16. ln2 weight folded
    into w1 (host), mean term via K=1 matmul fold, rstd2 at eviction.
    To hide the tail AllToAll (head 1), the first KSP m-groups run their
    EVEN c-tile partial sums early and spill them to SBUF (bf16); the odd
    contributions + fold are added when the AllToAll lands.
  DMAs are spread over sync/scalar/vector/gpsimd queues.
"""
import sys, math

sys.path.insert(0, "/opt/trn_rl_repo")

import numpy as np
import ml_dtypes

import concourse.bass as bass
import concourse.tile as tile
from concourse import bacc, mybir
from concourse.bass_utils import run_bass_kernel_spmd

# ---------------- constants (hardcoded problem shape) ----------------
P = 128
B, T, C = 2, 2048, 2048
H, D = 16, 128
R = 8                 # cores
HL = H // R           # heads per core
TOK = B * T // R      # own tokens per core
CT = C // P           # 16 c-tiles
NT = T // 512         # 4 t-blocks per batch
M1 = 4 * C            # 8192
MT = M1 // P          # 64 m-tiles
MG = 16               # m-groups of 4 m-tiles (512 cols) for matmul1
KSP = 4               # m-groups whose even partials are spilled early
RAW = 4               # QKV blocks evicted raw (before stats AG lands)
EPS = 1e-5
SCALE = 1.0 / math.sqrt(D)

F32 = mybir.dt.float32
BF16 = mybir.dt.bfloat16
AF = mybir.ActivationFunctionType
ALU = mybir.AluOpType

_CACHE = {}


def _pbc(t, n_free):
    """partition-broadcast AP over a 1-D dram tile view."""
    return bass.AP(tensor=t.tensor, offset=t.offset, ap=[[0, P], [1, n_free]])


def _row(ap1d):
    return ap1d.rearrange("(o t) -> o t", o=1)


def _build():
    nc = bacc.Bacc("TRN2", target_bir_lowering=False, debug=False, num_devices=R)

    # ---------------- I/O ----------------
    xT_d = nc.dram_tensor("xT", [B, C, T], BF16, kind="ExternalInput")
    xT_own_d = nc.dram_tensor("xT_own", [C, TOK], F32, kind="ExternalInput")
    xt_own_d = nc.dram_tensor("xt_own", [C, 512], BF16, kind="ExternalInput")
    wq_d = nc.dram_tensor("wq", [C, HL * D], BF16, kind="ExternalInput")
    wk_d = nc.dram_tensor("wk", [C, HL * D], BF16, kind="ExternalInput")
    wv_d = nc.dram_tensor("wv", [C, HL * D], BF16, kind="ExternalInput")
    nsq_d = nc.dram_tensor("nsq", [HL * D], BF16, kind="ExternalInput")
    nsk_d = nc.dram_tensor("nsk", [HL * D], BF16, kind="ExternalInput")
    nsv_d = nc.dram_tensor("nsv", [HL * D], BF16, kind="ExternalInput")
    w1_d = nc.dram_tensor("w1", [C, M1], BF16, kind="ExternalInput")
    ns1_d = nc.dram_tensor("ns1", [M1], BF16, kind="ExternalInput")
    w2r_d = nc.dram_tensor("w2r", [CT, MT, P, P], BF16, kind="ExternalInput")
    masks_d = nc.dram_tensor("masks", [P, P], BF16, kind="ExternalInput")
    out_d = nc.dram_tensor("outT", [C, TOK], F32, kind="ExternalOutput")

    with tile.TileContext(nc) as tc:
        with tc.tile_pool(name="dram", bufs=1, space="DRAM") as dram, \
             tc.tile_pool(name="psum", bufs=8, space="PSUM") as psum, \
             tc.tile_pool(name="singles", bufs=1) as singles:

            # internal DRAM
            warm_in = dram.tile([8], F32)
            warm_out = dram.tile([R, 8], F32)
            stats_loc = dram.tile([2, 512], F32)
            stats_g = dram.tile([R, 2, 512], F32)
            a2a_in = [dram.tile([R, P, 512], BF16, name=f"a2a_in{h}")
                      for h in range(HL)]
            a2a_out = [dram.tile([R, P, 512], BF16, name=f"a2a_out{h}")
                       for h in range(HL)]
            x1_spill = dram.tile([C, TOK], F32)
            mlp_stat_b = dram.tile([2, TOK], F32)

            def ps():
                return psum.tile([P, 512], F32, tag="ps", name="ps")

            # warmup collective: pays communicator init while QKV runs
            with tc.high_priority():
                nc.gpsimd.collective_compute(
                    "AllGather", ALU.bypass, replica_groups=[list(range(R))],
                    ins=[warm_in.opt()], outs=[warm_out.opt()])

            # small constants (vector engine; no DMA)
            eps_t = singles.tile([P, 1], F32)
            nc.vector.memset(eps_t, EPS)
            ones_bf = singles.tile([P, 1], BF16)
            nc.vector.memset(ones_bf, 1.0)
            ones_f32 = singles.tile([P, 1], F32)
            nc.vector.memset(ones_f32, 1.0)
            ones_row = singles.tile([1, P], BF16)   # lhsT for PE broadcasts
            nc.vector.memset(ones_row, 1.0)

            # =========== weight pool ===========
            _wpool_cm = tc.tile_pool(name="wqkv", bufs=1)
            wpool = _wpool_cm.__enter__()
            wq_t = wpool.tile([P, CT, HL * D], BF16)
            wk_t = wpool.tile([P, CT, HL * D], BF16)
            wv_t = wpool.tile([P, CT, HL * D], BF16)
            nsq_t = wpool.tile([1, HL * D], BF16)
            nsk_t = wpool.tile([1, HL * D], BF16)
            nsv_t = wpool.tile([1, HL * D], BF16)
            masks_t = wpool.tile([P, P], BF16)
            sv_rep_b = wpool.tile([P, HL * D], BF16)
            sv_rep = wpool.tile([P, HL * D], F32)
            nsq_cb = wpool.tile([P, HL], BF16)
            nsq_c = wpool.tile([P, HL], F32)
            nsk_cb = wpool.tile([P, HL], BF16)
            nsk_c = wpool.tile([P, HL], F32)

            def emit_preamble():
                nc.scalar.dma_start(nsq_t, _row(nsq_d.ap()))
                nc.scalar.dma_start(nsk_t, _row(nsk_d.ap()))
                nc.scalar.dma_start(nsv_t, _row(nsv_d.ap()))
                nc.scalar.dma_start(masks_t, masks_d.ap())
                nc.sync.dma_start(sv_rep_b, bass.AP(
                    tensor=nsv_d, offset=0, ap=[[0, P], [1, HL * D]]))
                nc.vector.tensor_copy(sv_rep, sv_rep_b)
                nc.sync.dma_start(nsq_cb,
                                  nsq_d.ap().rearrange("(hl d) -> d hl", d=P))
                nc.vector.tensor_copy(nsq_c, nsq_cb)
                nc.sync.dma_start(nsk_cb,
                                  nsk_d.ap().rearrange("(hl d) -> d hl", d=P))
                nc.vector.tensor_copy(nsk_c, nsk_cb)

            # per-block stat rows at partition base 0 (matmul rhs needs
            # base_partition == lhsT's, i.e. 0)
            murow_bf = [singles.tile([1, 512], BF16, name=f"murow{j}")
                        for j in range(R)]
            rstdrow_bf = [singles.tile([1, 512], BF16, name=f"rstdrow{j}")
                          for j in range(R)]
            # per-token columns for v evictions/fixups: [(j,ss)] -> [P,1]
            rstdc = singles.tile([P, R * 4], F32)
            murc = singles.tile([P, RAW * 4], F32)

            wqr = wq_d.ap().rearrange("(ko p) n -> p ko n", p=P)
            wkr = wk_d.ap().rearrange("(ko p) n -> p ko n", p=P)
            wvr = wv_d.ap().rearrange("(ko p) n -> p ko n", p=P)

            # =========== Phase B: QKV + integrated LN1 stats ===========
            with tc.tile_pool(name="qkvres", bufs=1) as qkvres, \
                 tc.tile_pool(name="xtp", bufs=12) as xtp, \
                 tc.tile_pool(name="stA", bufs=1) as stA, \
                 tc.tile_pool(name="reps", bufs=6) as reps, \
                 tc.tile_pool(name="tmps", bufs=4) as tmps, \
                 tc.tile_pool(name="attn", bufs=5) as attnp:

                # persistent qkv (bf16)
                qT = [[qkvres.tile([P, T], BF16, name=f"qT{h}{b}")
                       for b in range(B)] for h in range(HL)]
                kT = [[qkvres.tile([P, T], BF16, name=f"kT{h}{b}")
                       for b in range(B)] for h in range(HL)]
                vsb = [qkvres.tile([P, T // P, HL * D], BF16, name=f"v{b}")
                       for b in range(B)]

                pmu0 = ps()
                psq0 = ps()

                def gen_qkv(jlist):
                    for j in jlist:
                        b, tb = j // NT, j % NT
                        t0 = 512 * tb
                        raw = j < RAW

                        pq = [ps() for _ in range(HL)]
                        pk = [ps() for _ in range(HL)]
                        # 4 v chains share 2 banks: chain ss uses
                        # pv2[ss//2] cols [256*(ss%2) : 256*(ss%2)+256].
                        # Only chain with ss%2==0 uses start=True (bank
                        # clear); ss%2==1 relies on cleared has_written.
                        pv2 = [ps() for _ in range(2)]
                        for ko in range(CT):
                            if j == 0:
                                nc.scalar.dma_start(wq_t[:, ko], wqr[:, ko])
                                nc.scalar.dma_start(wv_t[:, ko], wvr[:, ko])
                                nc.sync.dma_start(wk_t[:, ko], wkr[:, ko])
                                xo = stA.tile([P, 512], BF16, tag="xo",
                                              name="xo", bufs=6)
                                nc.scalar.dma_start(
                                    xo, xt_own_d.ap()[ko * P:(ko + 1) * P, :])
                                sqx = stA.tile([P, 512], BF16, tag="sqx",
                                               name="sqx", bufs=4)
                                nc.vector.tensor_tensor(sqx, xo, xo, ALU.mult)
                                nc.tensor.matmul(pmu0[0:1, :], ones_bf, xo,
                                                 start=(ko == 0),
                                                 stop=(ko == CT - 1))
                                nc.tensor.matmul(psq0[0:1, :], ones_bf, sqx,
                                                 start=(ko == 0),
                                                 stop=(ko == CT - 1))
                            xt = xtp.tile([P, 512], BF16, tag="xt", name="xt")
                            nc.sync.dma_start(
                                xt,
                                xT_d.ap()[b, ko * P:(ko + 1) * P, t0:t0 + 512])
                            st_flag = ko == 0
                            for hl in range(HL):
                                nc.tensor.matmul(
                                    pq[hl], wq_t[:, ko, hl * D:(hl + 1) * D], xt,
                                    start=st_flag, stop=(raw and ko == CT - 1))
                                nc.tensor.matmul(
                                    pk[hl], wk_t[:, ko, hl * D:(hl + 1) * D], xt,
                                    start=st_flag, stop=(raw and ko == CT - 1))
                            yield
                            for ss in range(4):
                                nc.tensor.matmul(
                                    pv2[ss // 2][:, 256 * (ss % 2):
                                                 256 * (ss % 2) + 256],
                                    xt[:, ss * P:(ss + 1) * P], wv_t[:, ko, :],
                                    start=(st_flag and ss % 2 == 0),
                                    stop=(raw and ko == CT - 1),
                                    skip_group_check=True)
                            yield

                        if j == 0:
                            # finalize LN1 stats for own tokens
                            muA = stA.tile([1, 512], F32, tag="muA", name="muA")
                            nc.vector.tensor_scalar(muA, pmu0[0:1, :], 1.0 / C,
                                                    None, ALU.mult)
                            varA = stA.tile([1, 512], F32, tag="varA", name="varA")
                            nc.vector.tensor_scalar(varA, psq0[0:1, :], 1.0 / C,
                                                    None, ALU.mult)
                            musqA = stA.tile([1, 512], F32, tag="musqA",
                                             name="musqA")
                            nc.vector.tensor_tensor(musqA, muA, muA, ALU.mult)
                            nc.vector.tensor_tensor(varA, varA, musqA,
                                                    ALU.subtract)
                            rstdA = stA.tile([1, 512], F32, tag="rstdA",
                                             name="rstdA")
                            nc.scalar.activation(rstdA, varA, AF.Sqrt,
                                                 bias=eps_t[0:1])
                            nc.vector.reciprocal_approx_fast(out=rstdA, in_=rstdA)
                            murstdA = stA.tile([1, 512], F32, tag="murstdA",
                                               name="murstdA")
                            nc.vector.tensor_tensor(murstdA, muA, rstdA, ALU.mult)
                            nc.gpsimd.dma_start(_row(stats_loc[0, :]), rstdA)
                            nc.gpsimd.dma_start(_row(stats_loc[1, :]), murstdA)
                            nc.gpsimd.collective_compute(
                                "AllGather", ALU.bypass,
                                replica_groups=[list(range(R))],
                                ins=[stats_loc.opt()], outs=[stats_g.opt()])
                            # post-AG prefetch into SBUF (rows to base 0)
                            for jj in range(R):
                                rowf = tmps.tile([1, 512], F32, tag="rowf",
                                                 name="rowf", bufs=4)
                                nc.gpsimd.dma_start(rowf, _row(stats_g[jj, 0, :]))
                                nc.vector.tensor_copy(rstdrow_bf[jj], rowf)
                                rowf2 = tmps.tile([1, 512], F32, tag="rowf",
                                                  name="rowf2", bufs=4)
                                nc.gpsimd.dma_start(rowf2, _row(stats_g[jj, 1, :]))
                                nc.vector.tensor_copy(murow_bf[jj], rowf2)
                            for jj in range(R):
                                for ss in range(4):
                                    nc.scalar.dma_start(
                                        rstdc[:, jj * 4 + ss:jj * 4 + ss + 1],
                                        stats_g[jj, 0, ss * P:(ss + 1) * P]
                                        .rearrange("(p o) -> p o", o=1))
                            for jj in range(RAW):
                                for ss in range(4):
                                    nc.scalar.dma_start(
                                        murc[:, jj * 4 + ss:jj * 4 + ss + 1],
                                        stats_g[jj, 1, ss * P:(ss + 1) * P]
                                        .rearrange("(p o) -> p o", o=1))

                        if raw:
                            for hl in range(HL):
                                nc.vector.tensor_copy(
                                    qT[hl][b][:, t0:t0 + 512], pq[hl])
                                nc.vector.tensor_copy(
                                    kT[hl][b][:, t0:t0 + 512], pk[hl])
                            for ss in range(4):
                                nc.vector.tensor_copy(
                                    vsb[b][:, tb * 4 + ss, :],
                                    pv2[ss // 2][:, 256 * (ss % 2):
                                                 256 * (ss % 2) + 256])
                            yield
                            continue
                        # rank-1 LN fold: += (-colsum) x murstd  (K=1 matmul)
                        murow = murow_bf[j]
                        for hl in range(HL):
                            nc.tensor.matmul(
                                pq[hl], nsq_t[0:1, hl * D:(hl + 1) * D], murow,
                                start=False, stop=True)
                            nc.tensor.matmul(
                                pk[hl], nsk_t[0:1, hl * D:(hl + 1) * D], murow,
                                start=False, stop=True)
                        for ss in range(4):
                            nc.tensor.matmul(
                                pv2[ss // 2][:, 256 * (ss % 2):
                                             256 * (ss % 2) + 256],
                                murow[0:1, ss * P:(ss + 1) * P], nsv_t,
                                start=False, stop=(ss % 2 == 1),
                                skip_group_check=True)
                        # v evictions: per-partition rstd columns
                        for ss in range(4):
                            nc.vector.tensor_scalar(
                                vsb[b][:, tb * 4 + ss, :],
                                pv2[ss // 2][:, 256 * (ss % 2):
                                             256 * (ss % 2) + 256],
                                rstdc[:, 4 * j + ss:4 * j + ss + 1], None,
                                ALU.mult)
                        # q/k evictions: PE-broadcast rstd row -> SBUF copy
                        rb_ps = ps()
                        nc.tensor.matmul(rb_ps, ones_row, rstdrow_bf[j],
                                         start=True, stop=True)
                        rb_sb = reps.tile([P, 512], F32, tag="rbsb",
                                          name="rb_sb", bufs=3)
                        nc.vector.tensor_copy(rb_sb, rb_ps)
                        for hl in range(HL):
                            nc.vector.tensor_tensor(
                                qT[hl][b][:, t0:t0 + 512], pq[hl], rb_sb,
                                ALU.mult)
                            nc.vector.tensor_tensor(
                                kT[hl][b][:, t0:t0 + 512], pk[hl], rb_sb,
                                ALU.mult)
                        yield

                def emit_fixups():
                    # in-place LN1 fixups for raw blocks (wait on stats AG)
                    for j in range(RAW):
                        fb, ftb = j // NT, j % NT
                        ft0 = 512 * ftb
                        mur_bc = ps()
                        nc.tensor.matmul(mur_bc, ones_row, murow_bf[j],
                                         start=True, stop=True)
                        rstd_bc = ps()
                        nc.tensor.matmul(rstd_bc, ones_row, rstdrow_bf[j],
                                         start=True, stop=True)
                        for hl in range(HL):
                            for (tile_, s_col) in ((qT[hl][fb],
                                                    nsq_c[:, hl:hl + 1]),
                                                   (kT[hl][fb],
                                                    nsk_c[:, hl:hl + 1])):
                                tmp = tmps.tile([P, 512], F32, tag="fxt",
                                                name="fxt")
                                nc.vector.tensor_scalar(tmp, mur_bc, s_col, None,
                                                        ALU.mult)
                                nc.vector.tensor_tensor(
                                    tmp, tile_[:, ft0:ft0 + 512], tmp, ALU.add)
                                nc.vector.tensor_tensor(
                                    tile_[:, ft0:ft0 + 512], tmp, rstd_bc,
                                    ALU.mult)
                        for ss in range(4):
                            si = ftb * 4 + ss
                            tmpv = tmps.tile([P, HL * D], F32, tag="fxv",
                                             name="fxv")
                            nc.vector.tensor_scalar(
                                tmpv, sv_rep, murc[:, 4 * j + ss:4 * j + ss + 1],
                                None, ALU.mult)
                            nc.vector.tensor_tensor(tmpv, vsb[fb][:, si, :],
                                                    tmpv, ALU.add)
                            nc.vector.tensor_scalar(
                                vsb[fb][:, si, :], tmpv,
                                rstdc[:, 4 * j + ss:4 * j + ss + 1], None,
                                ALU.mult)

                def gen_attn(pairs, dve_den):
                    """Attention blocks. dve_den: softmax denominator via DVE
                    accumulation + one ones-matmul (cheap on PE, used when
                    interleaved with QKV); else per-si ones-matmuls on PE.
                    The AV matmul for si is deferred one step so the exp(si)
                    latency hides under interleaved work."""
                    for (hl, b) in pairs:
                        for tb in range(NT):
                            t0 = 512 * tb
                            n_s = 4 * (tb + 1)
                            pot = ps()
                            acc = None
                            pden = None if dve_den else ps()
                            pend = None
                            for si in range(n_s):
                                m = si - (n_s - 4)
                                w0 = max(m, 0) * P
                                pS = ps()
                                nc.tensor.matmul(
                                    pS[:, w0:512],
                                    kT[hl][b][:, si * P:(si + 1) * P],
                                    qT[hl][b][:, t0 + w0:t0 + 512],
                                    start=True, stop=True)
                                pt = attnp.tile([P, 512], BF16, tag="pt",
                                                name="pt", bufs=8)
                                nc.scalar.activation(pt[:, w0:512],
                                                     pS[:, w0:512], AF.Exp)
                                if m >= 0:
                                    nc.vector.tensor_tensor(
                                        pt[:, w0:w0 + P], pt[:, w0:w0 + P],
                                        masks_t, ALU.mult)
                                if dve_den:
                                    if si == 0:
                                        acc = attnp.tile([P, 512], F32,
                                                         tag="acc", name="acc",
                                                         bufs=2)
                                        nc.vector.tensor_copy(acc, pt)
                                    else:
                                        nc.vector.tensor_tensor(
                                            acc[:, w0:512], acc[:, w0:512],
                                            pt[:, w0:512], ALU.add)
                                if pend is not None:
                                    pend()

                                def mk(si=si, w0=w0, pt=pt):
                                    def f():
                                        nc.tensor.matmul(
                                            pot[:, w0:512],
                                            vsb[b][:, si, hl * D:(hl + 1) * D],
                                            pt[:, w0:512],
                                            start=(si == 0),
                                            stop=(si == n_s - 1))
                                        if not dve_den:
                                            nc.tensor.matmul(
                                                pden[0:1, w0:512], ones_bf,
                                                pt[:, w0:512],
                                                start=(si == 0),
                                                stop=(si == n_s - 1))
                                    return f
                                pend = mk()
                                yield
                            pend()
                            if dve_den:
                                acc_bf = attnp.tile([P, 512], BF16, tag="accbf",
                                                    name="accbf", bufs=2)
                                nc.vector.tensor_copy(acc_bf, acc)
                                pden = ps()
                                nc.tensor.matmul(pden[0:1, :], ones_bf, acc_bf,
                                                 start=True, stop=True)
                            den_r = attnp.tile([1, 512], F32, tag="dr",
                                               name="den_r")
                            nc.vector.reciprocal_approx_fast(out=den_r,
                                                             in_=pden[0:1, :])
                            den_rb = attnp.tile([1, 512], BF16, tag="drb",
                                                name="den_rb")
                            nc.vector.tensor_copy(den_rb, den_r)
                            rb = ps()
                            nc.tensor.matmul(rb, ones_row, den_rb,
                                             start=True, stop=True)
                            rb_s = attnp.tile([P, 512], F32, tag="rbs",
                                              name="rb_s", bufs=2)
                            nc.vector.tensor_copy(rb_s, rb)
                            ot = attnp.tile([P, 512], BF16, tag="ot", name="ot")
                            nc.vector.tensor_tensor(ot, pot, rb_s, ALU.mult)
                            nc.sync.dma_start(a2a_in[hl][NT * b + tb, :, :],
                                              ot)
                            yield

                # ---- Phase B part 1: batch-0 blocks (plain) ----
                for _ in gen_qkv([0]):
                    pass
                emit_preamble()
                for _ in gen_qkv([1, 2, 3]):
                    pass
                emit_fixups()

                # ---- interleave: batch-1 QKV x batch-0 attention ----
                gq = gen_qkv(range(NT, 2 * NT))
                ga = gen_attn([(0, 0), (1, 0)], dve_den=True)
                qn = an = 0
                q_done = a_done = False
                while not (q_done and a_done):
                    if not q_done:
                        try:
                            next(gq)
                            qn += 1
                        except StopIteration:
                            q_done = True
                    # hold attention back one QKV block (fixup margin);
                    # emit at most one attn step per qkv half-step to avoid
                    # back-to-back S matmuls stalling on exp
                    target = (an + 2) if q_done else max(0, qn - 33) * 0.9
                    if not a_done and an < target:
                        try:
                            next(ga)
                            an += 1
                        except StopIteration:
                            a_done = True

                # ---- tail attention: batch 1, heads interleaved ----
                ga0 = gen_attn([(0, 1)], dve_den=False)
                ga1 = gen_attn([(1, 1)], dve_den=True)
                n0 = n1 = 0
                d1 = False
                while True:
                    try:
                        next(ga0)
                        n0 += 1
                    except StopIteration:
                        break
                    while not d1 and n1 < n0 * 0.8:
                        try:
                            next(ga1)
                            n1 += 1
                        except StopIteration:
                            d1 = True
                nc.gpsimd.collective_compute(
                    "AllToAll", ALU.bypass,
                    replica_groups=[list(range(R))],
                    ins=[a2a_in[0].opt()], outs=[a2a_out[0].opt()])
                while not d1:
                    try:
                        next(ga1)
                        n1 += 1
                    except StopIteration:
                        d1 = True
                nc.gpsimd.collective_compute(
                    "AllToAll", ALU.bypass,
                    replica_groups=[list(range(R))],
                    ins=[a2a_in[1].opt()], outs=[a2a_out[1].opt()])

            _wpool_cm.__exit__(None, None, None)

            # =========== Phase E: MLP (token-sharded, bf16) ===========
            with tc.tile_pool(name="mlp_x1", bufs=2) as x1p, \
                 tc.tile_pool(name="mlp_sq", bufs=2) as sqp, \
                 tc.tile_pool(name="mlp_x1bf", bufs=1) as x1bfp, \
                 tc.tile_pool(name="mlp_g", bufs=1) as gp, \
                 tc.tile_pool(name="mlp_spill", bufs=1) as spillp, \
                 tc.tile_pool(name="mlp_w1", bufs=18) as w1p, \
                 tc.tile_pool(name="mlp_w2", bufs=2) as w2p, \
                 tc.tile_pool(name="mlp_z", bufs=4) as zp, \
                 tc.tile_pool(name="mlp_out", bufs=2) as outp:

                x1bf = [x1bfp.tile([P, TOK], BF16, name=f"x1bf{i}")
                        for i in range(CT)]
                evens = [2 * i for i in range(CT // 2)]
                odds = [2 * i + 1 for i in range(CT // 2)]

                pmu = ps()
                psq = ps()

                def build_x1(ct):
                    xo = x1p.tile([P, TOK], F32, tag="xo2", name="xo2")
                    nc.scalar.dma_start(xo, xT_own_d.ap()[ct * P:(ct + 1) * P, :])
                    at = x1p.tile([P, TOK], BF16, tag="at", name="at")
                    # evens on sync, odds on scalar: keeps each queue's head
                    # from blocking the other parity's A2A wait
                    eng = nc.sync if ct % 2 == 0 else nc.scalar
                    eng.dma_start(at, a2a_out[ct % 2][ct // 2])
                    x1 = x1p.tile([P, TOK], F32, tag="x1", name="x1")
                    nc.vector.tensor_tensor(x1, xo, at, ALU.add)
                    nc.vector.tensor_copy(x1bf[ct], x1)
                    nc.gpsimd.dma_start(x1_spill[ct * P:(ct + 1) * P, :], x1)

                def stats_mms(ct, idx):
                    sq2 = sqp.tile([P, TOK], BF16, tag="sq2", name="sq2")
                    nc.vector.tensor_tensor(sq2, x1bf[ct], x1bf[ct], ALU.mult)
                    nc.tensor.matmul(pmu[0:1, :], ones_bf, x1bf[ct],
                                     start=(idx == 0), stop=(idx == CT - 1))
                    nc.tensor.matmul(psq[0:1, :], ones_bf, sq2,
                                     start=(idx == 0), stop=(idx == CT - 1))

                w1tiles = {}

                def load_w1(mg, cts):
                    tiles = w1tiles.setdefault(mg, {})
                    for ct in cts:
                        w1t = w1p.tile([P, 512], BF16, tag="w1t", name="w1t")
                        nc.sync.dma_start(
                            w1t, w1_d.ap()[ct * P:(ct + 1) * P,
                                           mg * 512:(mg + 1) * 512])
                        tiles[ct] = w1t

                # w1 for the spill m-groups loads first (dep-free)
                for mg in range(KSP):
                    load_w1(mg, evens)

                # pass 1a: even x1 tiles (need only a2a head 0)
                for ct in evens:
                    build_x1(ct)

                # spill phase: even partial sums for the first KSP m-groups
                spill = [[spillp.tile([P, TOK], BF16, name=f"sp{mg}_{ml}")
                          for ml in range(4)] for mg in range(KSP)]
                for mg in range(KSP):
                    pgs = [ps() for _ in range(4)]
                    for ci, ct in enumerate(evens):
                        for ml in range(4):
                            nc.tensor.matmul(
                                pgs[ml],
                                w1tiles[mg][ct][:, ml * P:(ml + 1) * P],
                                x1bf[ct],
                                start=(ci == 0), stop=(ci == len(evens) - 1))
                    for ml in range(4):
                        nc.vector.tensor_copy(spill[mg][ml], pgs[ml])
                    if mg == 0:
                        for i, ct in enumerate(evens):
                            stats_mms(ct, i)

                # pass 1b: odd x1 tiles (need a2a head 1)
                for i, ct in enumerate(odds):
                    build_x1(ct)
                    stats_mms(ct, CT // 2 + i)

                # finalize stats: mu = pmu/C ; var = psq/C - mu^2
                mu2 = singles.tile([1, TOK], F32)
                nc.vector.tensor_scalar(mu2, pmu[0:1, :], 1.0 / C, None, ALU.mult)
                var2 = singles.tile([1, TOK], F32)
                nc.vector.tensor_scalar(var2, psq[0:1, :], 1.0 / C, None, ALU.mult)
                musq = singles.tile([1, TOK], F32)
                nc.vector.tensor_tensor(musq, mu2, mu2, ALU.mult)
                nc.vector.tensor_tensor(var2, var2, musq, ALU.subtract)
                rstd2 = singles.tile([1, TOK], F32)
                nc.scalar.activation(rstd2, var2, AF.Sqrt, bias=eps_t[0:1])
                nc.vector.reciprocal_approx_fast(out=rstd2, in_=rstd2)
                murow2 = singles.tile([1, TOK], BF16)
                nc.vector.tensor_copy(murow2, mu2)
                rstd2_bf = singles.tile([1, TOK], BF16)
                nc.vector.tensor_copy(rstd2_bf, rstd2)

                gT = gp.tile([P, MT, TOK], BF16)

                def ns1_row(mg):
                    ns1g = zp.tile([1, 512], BF16, tag="ns1g", name="ns1g", bufs=2)
                    nc.gpsimd.dma_start(
                        ns1g, _row(ns1_d.ap()[mg * 512:(mg + 1) * 512]))
                    return ns1g

                # rstd2 broadcast across partitions (PE ones-matmul), copied
                # to SBUF so the PSUM bank frees up for matmul1.
                rstd2_ps = ps()
                nc.tensor.matmul(rstd2_ps, ones_row, rstd2_bf,
                                 start=True, stop=True)
                rstd2_rep = singles.tile([P, TOK], F32)
                nc.vector.tensor_copy(rstd2_rep, rstd2_ps)

                # matmul1 for spilled m-groups: odds + fold, then add spill
                for mg in range(KSP):
                    ns1g = ns1_row(mg)
                    load_w1(mg, odds)
                    pgs = [ps() for _ in range(4)]
                    for ci, ct in enumerate(odds):
                        for ml in range(4):
                            nc.tensor.matmul(
                                pgs[ml],
                                w1tiles[mg][ct][:, ml * P:(ml + 1) * P],
                                x1bf[ct],
                                start=(ci == 0), stop=False)
                    for ml in range(4):
                        mt = mg * 4 + ml
                        nc.tensor.matmul(
                            pgs[ml], ns1g[0:1, ml * P:(ml + 1) * P],
                            murow2, start=False, stop=True)
                        ztf = zp.tile([P, TOK], F32, tag="ztf", name="ztf", bufs=2)
                        nc.vector.tensor_tensor(ztf, pgs[ml], spill[mg][ml],
                                                ALU.add)
                        zt = zp.tile([P, TOK], BF16, tag="zt", name="zt")
                        nc.vector.tensor_tensor(zt, ztf, rstd2_rep, ALU.mult)
                        nc.scalar.activation(gT[:, mt, :], zt,
                                             AF.Gelu_apprx_tanh)
                    del w1tiles[mg]

                # matmul1 for remaining m-groups (all 16 c-tiles)
                for mg in range(KSP, MG):
                    ns1g = ns1_row(mg)
                    load_w1(mg, evens + odds)
                    pgs = [ps() for _ in range(4)]
                    for ci, ct in enumerate(evens + odds):
                        for ml in range(4):
                            nc.tensor.matmul(
                                pgs[ml],
                                w1tiles[mg][ct][:, ml * P:(ml + 1) * P],
                                x1bf[ct],
                                start=(ci == 0), stop=False)
                    for ml in range(4):
                        mt = mg * 4 + ml
                        nc.tensor.matmul(
                            pgs[ml], ns1g[0:1, ml * P:(ml + 1) * P],
                            murow2, start=False, stop=True)
                        zt = zp.tile([P, TOK], BF16, tag="zt", name="zt")
                        nc.vector.tensor_tensor(zt, pgs[ml], rstd2_rep,
                                                ALU.mult)
                        nc.scalar.activation(gT[:, mt, :], zt,
                                             AF.Gelu_apprx_tanh)
                    del w1tiles[mg]

                # matmul2 + residual -> outT
                for co in range(CT):
                    w2t = w2p.tile([P, MT, P], BF16, tag="w2t", name="w2t")
                    nc.scalar.dma_start(
                        w2t, w2r_d.ap()[co].rearrange("mo p c -> p mo c"))
                    po = ps()
                    for mt in range(MT):
                        nc.tensor.matmul(po, w2t[:, mt, :], gT[:, mt, :],
                                         start=(mt == 0), stop=(mt == MT - 1))
                    x1r = x1p.tile([P, TOK], F32, tag="x1o", name="x1o")
                    nc.scalar.dma_start(x1r, x1_spill[co * P:(co + 1) * P, :])
                    ot2 = outp.tile([P, TOK], F32, tag="ot2", name="ot2")
                    nc.vector.tensor_tensor(ot2, po, x1r, ALU.add)
                    nc.scalar.dma_start(out_d.ap()[co * P:(co + 1) * P, :], ot2)

    nc.compile()
    return nc


def _host_prep(x, w_qkv, w1, w2, ln_w):
    x = np.asarray(x, dtype=np.float32)
    w_qkv = np.asarray(w_qkv, dtype=np.float32)
    w1 = np.asarray(w1, dtype=np.float32)
    w2 = np.asarray(w2, dtype=np.float32)
    ln_w = np.asarray(ln_w, dtype=np.float32)

    xT = np.ascontiguousarray(x.transpose(0, 2, 1))            # [B, C, T]
    xT_bf = xT.astype(ml_dtypes.bfloat16)

    Wq = (ln_w[:, None] * w_qkv[:, 0 * C:1 * C]) * SCALE
    Wk = ln_w[:, None] * w_qkv[:, 1 * C:2 * C]
    Wv = ln_w[:, None] * w_qkv[:, 2 * C:3 * C]
    nsq_full = -Wq.sum(0, dtype=np.float64).astype(np.float32)
    nsk_full = -Wk.sum(0, dtype=np.float64).astype(np.float32)
    nsv_full = -Wv.sum(0, dtype=np.float64).astype(np.float32)

    w1s = ln_w[:, None] * w1
    w1_bf = w1s.astype(ml_dtypes.bfloat16)
    ns1 = -w1s.sum(0, dtype=np.float64).astype(np.float32)
    # w2 reordered: [CT, MT, P(m), P(c)]
    w2r = np.ascontiguousarray(
        w2.reshape(MT, P, CT, P).transpose(2, 0, 1, 3)).astype(ml_dtypes.bfloat16)

    masks = (np.arange(P)[None, :] >= np.arange(P)[:, None]).astype(
        np.float32).astype(ml_dtypes.bfloat16)

    in_maps = []
    for r in range(R):
        cs = slice(256 * r, 256 * (r + 1))
        b_own, tb_own = r // NT, r % NT
        in_maps.append({
            "xT": xT_bf,
            "xT_own": np.ascontiguousarray(
                xT[b_own][:, 512 * tb_own: 512 * (tb_own + 1)]),
            "xt_own": np.ascontiguousarray(
                xT_bf[b_own][:, 512 * tb_own: 512 * (tb_own + 1)]),
            "wq": np.ascontiguousarray(Wq[:, cs]).astype(ml_dtypes.bfloat16),
            "wk": np.ascontiguousarray(Wk[:, cs]).astype(ml_dtypes.bfloat16),
            "wv": np.ascontiguousarray(Wv[:, cs]).astype(ml_dtypes.bfloat16),
            "nsq": np.ascontiguousarray(nsq_full[cs]).astype(ml_dtypes.bfloat16),
            "nsk": np.ascontiguousarray(nsk_full[cs]).astype(ml_dtypes.bfloat16),
            "nsv": np.ascontiguousarray(nsv_full[cs]).astype(ml_dtypes.bfloat16),
            "w1": w1_bf,
            "ns1": ns1.astype(ml_dtypes.bfloat16),
            "w2r": w2r,
            "masks": masks,
        })
    return in_maps


def get_nc():
    if "nc" not in _CACHE:
        _CACHE["nc"] = _build()
    return _CACHE["nc"]


def run(in_maps, **kw):
    nc = get_nc()
    return run_bass_kernel_spmd(nc, in_maps, core_ids=list(range(R)), **kw)


def kernel(x, w_qkv, w1, w2, ln_w, **kw_unused):
    in_maps = _host_prep(x, w_qkv, w1, w2, ln_w)
    res = run(in_maps)
    out_flat = np.empty((B * T, C), np.float32)
    for r in range(R):
        out_flat[TOK * r: TOK * (r + 1)] = res.results[r]["outT"].T
    return out_flat.reshape(B, T, C)
